# revision 28
# baseline (speedup 1.0000x reference)
"""Trainium2 Bass kernel for nn_Decoder_22196390985918 (SPADE-style decoder).

Sharding: 8 cores = (batch b in 0..3) x (H-half in 0..1). Each core computes
out[b, :, h0:h0+64, :] for h0 = 64*(core%2).

The [B, 512, H, W] "middle" tensor (masked scatter of per-region style
vectors mu[b,j,:]) is never materialized: conv(middle) collapses to a conv
over the 5 one-hot region masks sel_j with per-batch tap tables
G[j, cc, tap] = sum_k Wconv[cc, k, tap] * mu[b, j, k], i.e. one K=45 matmul
per output tile on top of the 9 K=128 SPADE tap matmuls.

All heavy matmul operands are bf16; weight transposes and the sigmoid
blending factors are folded on the host; every large DMA is a plain 2D
transfer with per-partition-contiguous spans. The PE stream is ordered to
avoid idling while weights land: the SPADE tap matmuls for chunks 0-2 run
first (their psum groups stay open), then mu (fc bias folded in as a K=1
matmul row), the G tables, a 9-matmul partition-shuffle that builds selG in
psum, and only then the selG "close" matmuls. gamma/beta leave PSUM via two
half-height ACTs (the beta ACT reads PSUM partitions 64:128 and writes
0:64). Relu/compare epilogues of the aux chunks run on DVE/Pool via
tensor_scalar (is_lt / add+max) to keep the ACT engine off the critical
path. Instance-norm stats come from a bf16 copy of the full image.
"""
import os as _os

import numpy as np
import ml_dtypes

import concourse.bacc as bacc
import concourse.bass as bass
import concourse.mybir as mybir
import concourse.tile as tile
from concourse.bass_utils import run_bass_kernel_spmd

dt = mybir.dt
F32 = dt.float32
BF16 = dt.bfloat16
AF = mybir.ActivationFunctionType
ALU = mybir.AluOpType
NPBF = ml_dtypes.bfloat16

B, C, H, W, F, L, NH = 4, 64, 128, 128, 5, 512, 128
GW = 130                    # padded grid width  (image col = grid col - 1)
SR = 66                     # seg/sel/actv grid rows (image row = h0 - 1 + r)
MR = 68                     # mask grid rows (image row = h0 - 2 + r)
SEG_N = SR * GW             # 8580
MASK_N = MR * GW            # 8840
ROWS = 64                   # output rows per core
NCH = 16                    # main conv chunks (4 rows x 128 cols, N=512)
ACH = 22                    # shared conv chunks (3 rows x 128 cols, N=384)
NCORES = 8
CNTC = 17                   # cnt/sel chunks of 512 cols over SEG_N
PKF = 25                    # f32 const pack cols
# bf16 pack: u5(45) | sswT(128) | codesT(20) | zeros(132)
PKB = 325
# small pack on 8 partitions: fcb rows(20*128) | Et(9*45) | one
PKE_ET = 20 * 128
PKE = PKE_ET + 9 * 45 + 1


def _build_nc():
    lvl = int(_os.environ.get("KSEC", "9"))
    nc = bacc.Bacc()

    # ---- per-core DRAM inputs -------------------------------------------
    xq_d = nc.dram_tensor("xq", [128, ROWS * W], BF16, kind="ExternalInput")
    segg = nc.dram_tensor("segg", [F, SEG_N + 264], BF16, kind="ExternalInput")
    maskg = nc.dram_tensor("maskg", [3, MASK_N + 264], BF16,
                           kind="ExternalInput")
    fcwT_d = nc.dram_tensor("fcwT", [128, F * 4 * L], BF16,
                            kind="ExternalInput")
    wct_d = nc.dram_tensor("wct", [L, 9 * 128], BF16, kind="ExternalInput")
    spT_d = nc.dram_tensor("spT", [NH, 9 * 128], BF16, kind="ExternalInput")
    pkf_d = nc.dram_tensor("pkf", [128, PKF], F32, kind="ExternalInput")
    pkb_d = nc.dram_tensor("pkb", [128, PKB], BF16, kind="ExternalInput")
    pke_d = nc.dram_tensor("pke", [8, PKE], BF16, kind="ExternalInput")
    out_d = nc.dram_tensor("out", [C, NCH, 512], F32, kind="ExternalOutput")

    segp = segg[:].ap[0][0]     # dram row stride (elements)
    maskp = maskg[:].ap[0][0]

    with tile.TileContext(nc) as tc:
        with (
            tc.tile_pool(name="const", bufs=1) as cst,
            tc.tile_pool(name="gg", bufs=3) as ggp,
            tc.tile_pool(name="bb", bufs=3) as bbp,
            tc.tile_pool(name="ot", bufs=3) as otp,
            tc.tile_pool(name="pmain", bufs=4, space="PSUM") as pmain,
            tc.tile_pool(name="paux", bufs=2, space="PSUM") as paux,
            tc.tile_pool(name="gpsp", bufs=2, space="PSUM") as gpsp,
        ):
            # ---- const packs (first on sync queue) ----------------------
            pkf = cst.tile([128, PKF], F32)
            nc.sync.dma_start(out=pkf[:], in_=pkf_d[:])
            pkb = cst.tile([128, PKB], BF16)
            nc.sync.dma_start(out=pkb[:], in_=pkb_d[:])
            pke = cst.tile([8, PKE], BF16)
            nc.sync.dma_start(out=pke[:], in_=pke_d[:])
            bias1g = pkf[0:64, 20:21]
            bias1b = pkf[0:64, 21:22]
            ssb_t = pkf[:, 22:23]
            ones45 = pkf[0:45, 19:20]
            hal_t = pkf[:, 23:25]
            u5r = pkb[0:45, 0:45]
            zsb = pkb[:, 193:325]
            eps_t = pkf[0:64, 18:19]
            ones_bf = pke[0:1, PKE - 1:PKE]
            sswT = pkb[0:27, 45:173]
            codesT = pkb[:, 173:193].rearrange("p (l j) -> p l j", l=4)

            # ---- fcwT: plain 2D loads, j0/j2/j4 scalar, j1/j3 sync ------
            ftall = cst.tile([128, F, 4, L], BF16)
            ftq = {0: nc.scalar, 1: nc.sync, 2: nc.scalar, 3: nc.sync,
                   4: nc.gpsimd}
            def load_ft(j):
                ftq[j].dma_start(
                    out=ftall[:, j, :, :].rearrange("p l k -> p (l k)"),
                    in_=fcwT_d[:, j * 4 * L:(j + 1) * 4 * L])



            # grids spread across all three queues for parallel arrival
            sel45 = cst.tile([45, SEG_N], BF16)
            selq = [nc.sync, nc.scalar, nc.gpsimd]
            for ty in range(3):
                src = bass.AP(tensor=segg[:].tensor, offset=ty * GW,
                              ap=[[1, 3], [segp, F], [1, SEG_N]])
                selq[ty].dma_start(out=sel45[15 * ty:15 * ty + 15, :],
                                   in_=src)
            mask27 = cst.tile([27, MASK_N], BF16)
            mq = [nc.scalar, nc.gpsimd, nc.gpsimd]
            for ty in range(3):
                src = bass.AP(tensor=maskg[:].tensor, offset=ty * GW,
                              ap=[[1, 3], [maskp, 3], [1, MASK_N]])
                mq[ty].dma_start(out=mask27[9 * ty:9 * ty + 9, :], in_=src)
            spT_f = cst.tile([128, 9 * 128], BF16)
            nc.sync.dma_start(out=spT_f[:], in_=spT_d[:])
            spT = spT_f[:].rearrange("p (t c) -> p t c", t=9)
            for j in range(F):
                load_ft(j)
            wct_sb = cst.tile([128, 4, 9 * 128], BF16)
            wq = [nc.scalar, nc.gpsimd, nc.gpsimd, nc.sync]
            for kb in range(4):
                wq[kb].dma_start(
                    out=wct_sb[:, kb, :],
                    in_=wct_d[kb * 128:(kb + 1) * 128, :])
            wcts = [wct_sb[:, kb, :].rearrange("p (t c) -> p t c", t=9)
                    for kb in range(4)]
            # x lives once in SBUF as bf16 [128, 8192]: own half on
            # partitions 0:64 (stats + epilogue), other half on 64:128
            # (stats only; halves merged in closed form). 4 pieces on
            # queue tails.
            xq = cst.tile([128, ROWS * W], BF16)
            xqq = [nc.scalar, nc.gpsimd, nc.sync, nc.scalar]
            for h in range(4):
                xqq[h].dma_start(out=xq[:, h * 2048:(h + 1) * 2048],
                                 in_=xq_d[:, h * 2048:(h + 1) * 2048])

            # ---- aux tiles + emitters -----------------------------------
            t_sb = cst.tile([45, SEG_N], BF16)
            actv = cst.tile([NH, SR, GW], BF16)
            # zero border cols 0 and 129 of actv (vector, only needs zsb)
            bord = actv[:, :, 0:1]
            nc.vector.tensor_copy(
                bass.AP(tensor=bord.tensor, offset=bord.offset,
                        ap=[bord.ap[0], [GW, SR], [GW - 1, 2]]),
                zsb.rearrange("p (a b) -> p a b", a=SR))
            m3 = mask27[:].rearrange("p (r c) -> p r c", c=GW)
            s3 = sel45[:].rearrange("p (r c) -> p r c", c=GW)

            segchunks = []
            off = 0
            while off < SEG_N:
                n = min(512, SEG_N - off)
                segchunks.append((off, n))
                off += n

            def cnt_chunk(c):
                off, n = segchunks[c]
                pc = paux.tile([45, 512], F32, tag="aux", name=f"cnt{c}")
                nc.tensor.matmul(pc[:, 0:n], u5r, sel45[:, off:off + n],
                                 start=True, stop=True)
                # t = relu(1 - cnt); sel *= t  (exact in bf16)
                nc.scalar.activation(t_sb[:, off:off + n], pc[:, 0:n],
                                     AF.Relu, bias=ones45, scale=-1.0)
                nc.vector.tensor_mul(sel45[:, off:off + n],
                                     sel45[:, off:off + n],
                                     t_sb[:, off:off + n])

            def shared_chunk(a):
                r = 3 * a
                psh = paux.tile([NH, 3, 128], F32, tag="aux", name=f"sh{a}")
                nc.tensor.matmul(psh[:], sswT, m3[:, r:r + 3, 0:128],
                                 start=True, stop=True)
                nc.scalar.activation(actv[:, r:r + 3, 1:129], psh[:],
                                     AF.Relu, bias=ssb_t, scale=1.0)
                if a == 0:
                    nc.vector.tensor_scalar_mul(actv[:, 0, :], actv[:, 0, :],
                                                hal_t[:, 0:1])
                elif a == ACH - 1:
                    nc.vector.tensor_scalar_mul(actv[:, SR - 1, :],
                                                actv[:, SR - 1, :],
                                                hal_t[:, 1:2])

            cnt_done = [0]
            sh_done = [0]

            def aux_for(i):
                need_cnt = min(CNTC, (520 * i + 518) // 512 + 1)
                need_sh = min(ACH, (4 * i + 6) // 3 + 1)
                while cnt_done[0] < need_cnt:
                    cnt_chunk(cnt_done[0])
                    cnt_done[0] += 1
                while sh_done[0] < need_sh:
                    shared_chunk(sh_done[0])
                    sh_done[0] += 1

            # ---- main conv pieces ---------------------------------------
            pms = {}

            def taps_chunk(i):
                pm = pmain.tile([128, 4, 128], F32, tag="pm", name=f"pm{i}")
                pms[i] = pm
                for t in range(9):
                    ty, tx = divmod(t, 3)
                    nc.tensor.matmul(
                        pm[:], spT[:, t, :],
                        actv[:, 4 * i + ty:4 * i + ty + 4, tx:tx + 128],
                        start=(t == 0), stop=False)

            def close_chunk(i, selG):
                nc.tensor.matmul(pms[i][:], selG,
                                 s3[:, 4 * i:4 * i + 4, 0:128],
                                 start=False, stop=True)

            def epi_chunk(i, rstd, nbias):
                pm = pms.pop(i)
                pmf = pm[:].rearrange("p t c -> p (t c)")
                gg = ggp.tile([C, 512], F32, tag="gg", name=f"gg{i}")
                nc.scalar.activation(gg[:], pmf[0:64, :], AF.Identity,
                                     bias=bias1g, scale=1.0)
                bb = bbp.tile([C, 512], F32, tag="bb", name=f"bb{i}")
                nc.scalar.activation(bb[:], pmf[64:128, :], AF.Identity,
                                     bias=bias1b, scale=1.0)
                xnt = otp.tile([C, 512], F32, tag="ot", name=f"xnt{i}")
                nc.gpsimd.tensor_scalar(xnt[:],
                                        xq[0:64, i * 512:(i + 1) * 512],
                                        rstd, nbias,
                                        op0=ALU.mult, op1=ALU.add)
                nc.gpsimd.tensor_mul(xnt[:], xnt[:], gg[:])
                nc.vector.tensor_add(xnt[:], xnt[:], bb[:])
                oeng = nc.sync if i % 2 == 0 else nc.scalar
                oeng.dma_start(out=out_d[:, i, :], in_=xnt[:])

            # ---- PE stream ----------------------------------------------
            # cnt chunks first (need only sel45, the first grid to land),
            # then shared (mask27), taps 0-2, then the cnt tail
            for c in range(9):
                cnt_chunk(c)
            for a in range(5):
                shared_chunk(a)
            sh_done[0] = 5
            for i in range(3):
                taps_chunk(i)
            for c in range(9, CNTC):
                cnt_chunk(c)
            cnt_done[0] = CNTC


            # mu: 100 small matmuls (fc bias folded in as K=1 row)
            pz = gpsp.tile([128, 4, F], F32, tag="gps", name="pz")
            for j in range(F):
                for kb in range(4):
                    for lb in range(4):
                        nc.tensor.matmul(
                            pz[:, kb, j:j + 1],
                            ftall[:, j, lb, kb * 128:(kb + 1) * 128],
                            codesT[:, lb, j:j + 1],
                            start=(lb == 0), stop=False)
                    fcbcol = (j * 4 + kb) * 128
                    nc.tensor.matmul(pz[:, kb, j:j + 1],
                                     pke[0:1, fcbcol:fcbcol + 128],
                                     ones_bf,
                                     start=False, stop=True)
            muT = cst.tile([128, 4, F], BF16)
            nc.scalar.activation(muT[:], pz[:], AF.Relu)

            # G matmuls (group-sequential so gpsp needs only 2 banks)
            gstage = cst.tile([F, 9, 128], BF16)
            for g in range(3):
                gp = gpsp.tile([F, 3, 128], F32, tag="gps", name=f"gps{g}")
                for kb in range(4):
                    nc.tensor.matmul(gp[:], muT[:, kb, :],
                                     wcts[kb][:, 3 * g:3 * g + 3, :],
                                     start=(kb == 0), stop=(kb == 3))
                nc.scalar.activation(gstage[:, 3 * g:3 * g + 3, :],
                                     gp[:], AF.Copy)
            # selG[5t+j, cc] = gstage[j, t, cc] via 9 accumulating
            # partition-shuffle matmuls (lhsT Et[j, 5t+j] = 1)
            selG_ps = gpsp.tile([45, 128], F32, tag="gps", name="selG_ps")
            for t in range(9):
                etcol = PKE_ET + 45 * t
                nc.tensor.matmul(selG_ps[:], pke[0:5, etcol:etcol + 45],
                                 gstage[:, t, :],
                                 start=(t == 0), stop=(t == 8))
            selG_t = cst.tile([45, 128], BF16)
            nc.scalar.activation(selG_t[:], selG_ps[:], AF.Copy)
            selG = selG_t[:]

            # instance-norm stats: both image halves at once on 128
            # partitions, then closed-form merge of the two halves
            stats_t = cst.tile([128, 16, 6], F32)
            for q in range(16):
                nc.vector.bn_stats(out=stats_t[:, q, :],
                                   in_=xq[:, q * 512:(q + 1) * 512])
            mv = cst.tile([128, 2], F32)
            nc.vector.bn_aggr(out=mv[:], in_=stats_t[:])
            mvhi = cst.tile([C, 2], F32)
            nc.sync.dma_start(out=mvhi[:], in_=mv[64:128, :])
            # mean = (m0+m1)/2 ; var = (v0+v1)/2 + ((m0-m1)/2)^2
            mean = cst.tile([C, 1], F32)
            nc.vector.tensor_add(mean[:], mv[0:64, 0:1], mvhi[:, 0:1])
            nc.vector.tensor_scalar_mul(mean[:], mean[:], 0.5)
            md = cst.tile([C, 1], F32)
            nc.vector.tensor_sub(md[:], mv[0:64, 0:1], mvhi[:, 0:1])
            nc.vector.tensor_scalar_mul(md[:], md[:], 0.5)
            nc.vector.tensor_mul(md[:], md[:], md[:])
            var = cst.tile([C, 1], F32)
            nc.vector.tensor_add(var[:], mv[0:64, 1:2], mvhi[:, 1:2])
            nc.vector.tensor_scalar(var[:], var[:], 0.5, None, op0=ALU.mult)
            nc.vector.tensor_add(var[:], var[:], md[:])
            sd = cst.tile([C, 1], F32)
            nc.scalar.activation(sd[:], var[:], AF.Sqrt,
                                 bias=eps_t, scale=1.0)
            rstd = cst.tile([C, 1], F32)
            nc.vector.reciprocal(rstd[:], sd[:])
            nbias = cst.tile([C, 1], F32)
            nc.vector.tensor_mul(nbias[:], mean[:], rstd[:])
            nc.vector.tensor_scalar_mul(nbias[:], nbias[:], -1.0)

            if lvl >= 6:
                for i in range(3):
                    close_chunk(i, selG)
                epi_chunk(0, rstd[:], nbias[:])
                for i in range(3, NCH):
                    aux_for(i)
                    taps_chunk(i)
                    close_chunk(i, selG)
                    epi_chunk(i - 2, rstd[:], nbias[:])
                while cnt_done[0] < CNTC:
                    cnt_chunk(cnt_done[0])
                    cnt_done[0] += 1
                while sh_done[0] < ACH:
                    shared_chunk(sh_done[0])
                    sh_done[0] += 1
                epi_chunk(NCH - 2, rstd[:], nbias[:])
                epi_chunk(NCH - 1, rstd[:], nbias[:])

    nc.finalize()
    return nc


_NC = None


def _make_in_maps(inputs):
    x = np.asarray(inputs["x"], dtype=np.float32)
    segmap = np.asarray(inputs["segmap"], dtype=np.float32)
    codes_vector = np.asarray(inputs["codes_vector"], dtype=np.float32)
    mask = np.asarray(inputs["mask"], dtype=np.float32)
    fc_w = np.asarray(inputs["fc_w"], dtype=np.float32)
    fc_b = np.asarray(inputs["fc_b"], dtype=np.float32)
    conv_gamma_w = np.asarray(inputs["conv_gamma_w"], dtype=np.float32)
    conv_gamma_b = np.asarray(inputs["conv_gamma_b"], dtype=np.float32)
    conv_beta_w = np.asarray(inputs["conv_beta_w"], dtype=np.float32)
    conv_beta_b = np.asarray(inputs["conv_beta_b"], dtype=np.float32)
    spade_shared_w = np.asarray(inputs["spade_shared_w"], dtype=np.float32)
    spade_shared_b = np.asarray(inputs["spade_shared_b"], dtype=np.float32)
    spade_gamma_w = np.asarray(inputs["spade_gamma_w"], dtype=np.float32)
    spade_gamma_b = np.asarray(inputs["spade_gamma_b"], dtype=np.float32)
    spade_beta_w = np.asarray(inputs["spade_beta_w"], dtype=np.float32)
    spade_beta_b = np.asarray(inputs["spade_beta_b"], dtype=np.float32)
    blending_gamma = np.asarray(inputs["blending_gamma"], dtype=np.float32)
    blending_beta = np.asarray(inputs["blending_beta"], dtype=np.float32)

    ga = 1.0 / (1.0 + np.exp(-float(blending_gamma[0])))
    ba = 1.0 / (1.0 + np.exp(-float(blending_beta[0])))

    # combined conv weights, blend folded in, transposed to lhsT layouts
    wc = np.concatenate([ga * conv_gamma_w, ba * conv_beta_w], axis=0)
    wct = wc.transpose(1, 2, 3, 0).reshape(L, 9 * 128)        # [k,(t,cc)]
    sp = np.concatenate([(1.0 - ga) * spade_gamma_w,
                         (1.0 - ba) * spade_beta_w], axis=0)
    spT = sp.transpose(1, 2, 3, 0).reshape(NH, 9 * 128)       # [nh,(t,cc)]
    sswT = spade_shared_w.transpose(0, 2, 3, 1).reshape(NH, 27).T  # [27,nh]
    # fcwT host layout: [p(128), j, lb, k] so each partition's data is one
    # contiguous DRAM span (descriptor-cheap 2D DMA)
    fcwT = np.ascontiguousarray(
        fc_w.transpose(0, 2, 1).reshape(F, 4, 128, L)
        .transpose(2, 0, 1, 3).reshape(128, F * 4 * L))

    # f32 const pack: (20 unused) | bias1g | bias1b | ssb | hal(2)
    pkf = np.zeros((128, PKF), np.float32)
    pkf[:, 19] = 1.0
    pkf[0:64, 20] = ga * conv_gamma_b + (1.0 - ga) * spade_gamma_b + 1.0
    pkf[0:64, 21] = ba * conv_beta_b + (1.0 - ba) * spade_beta_b
    pkf[:, 22] = spade_shared_b
    u5 = np.kron(np.eye(9, dtype=np.float32),
                 np.tril(np.ones((F, F), np.float32), -1))
    # Et[t]: [5, 45] with Et[j, 5t+j] = 1 (partition shuffle for selG)
    et = np.zeros((9, F, 45), np.float32)
    for t in range(9):
        for j in range(F):
            et[t, j, F * t + j] = 1.0

    shared = {
        "fcwT": fcwT.astype(NPBF),
        "wct": np.ascontiguousarray(wct).astype(NPBF),
        "spT": np.ascontiguousarray(spT).astype(NPBF),
    }

    in_maps = []
    for c in range(NCORES):
        b, half = divmod(c, 2)
        h0 = half * ROWS
        segp = np.zeros((F, SEG_N + 264), NPBF)
        segp2 = np.zeros((F, SR, GW), np.float32)
        r_lo, r_hi = h0 - 1, h0 + ROWS + 1  # exclusive
        s_lo, s_hi = max(r_lo, 0), min(r_hi, H)
        segp2[:, s_lo - r_lo:s_hi - r_lo, 1:129] = segmap[b, :, s_lo:s_hi, :]
        segp[:, 0:SEG_N] = segp2.reshape(F, -1).astype(NPBF)
        maskp = np.zeros((3, MASK_N + 264), NPBF)
        maskp2 = np.zeros((3, MR, GW), np.float32)
        m_lo, m_hi = h0 - 2, h0 + ROWS + 2
        ms_lo, ms_hi = max(m_lo, 0), min(m_hi, H)
        maskp2[:, ms_lo - m_lo:ms_hi - m_lo, 1:129] = mask[b, :, ms_lo:ms_hi, :]
        maskp[:, 0:MASK_N] = maskp2.reshape(3, -1).astype(NPBF)
        pkfc = pkf.copy()
        pkfc[:, 23] = 0.0 if h0 == 0 else 1.0
        pkfc[:, 24] = 0.0 if h0 + ROWS == H else 1.0
        pkb = np.zeros((128, PKB), NPBF)
        pkb[0:45, 0:45] = u5.astype(NPBF)
        pkb[0:27, 45:173] = sswT.astype(NPBF)
        pkb[:, 173:193] = (codes_vector[b].T.reshape(4, 128, F)
                           .transpose(1, 0, 2).reshape(128, 20).astype(NPBF))
        pke = np.zeros((8, PKE), NPBF)
        pke[0, 0:PKE_ET] = fc_b.reshape(F * 4 * 128).astype(NPBF)
        pke[0:F, PKE_ET:PKE - 1] = et.transpose(1, 0, 2).reshape(F, 9 * 45)
        pke[0, PKE - 1] = 1.0
        oh0 = ROWS - h0
        xsw = np.concatenate([x[b, :, h0:h0 + ROWS, :].reshape(C, ROWS * W),
                              x[b, :, oh0:oh0 + ROWS, :].reshape(C, ROWS * W)],
                             axis=0)
        in_maps.append(dict(
            shared,
            xq=np.ascontiguousarray(xsw).astype(NPBF),
            pkf=pkfc,
            pkb=pkb,
            pke=pke,
            segg=np.ascontiguousarray(segp),
            maskg=np.ascontiguousarray(maskp),
        ))
    return in_maps


def kernel(**inputs):
    global _NC
    if _NC is None:
        _NC = _build_nc()
    in_maps = _make_in_maps(inputs)
    res = run_bass_kernel_spmd(_NC, in_maps, list(range(NCORES)))

    out = np.empty((B, C, H, W), np.float32)
    for c in range(NCORES):
        b, half = divmod(c, 2)
        h0 = half * ROWS
        out[b, :, h0:h0 + ROWS, :] = res.results[c]["out"].reshape(C, ROWS, W)
    return out


# revision 29
# speedup vs baseline: 1.1313x; 1.1313x over previous
"""Trainium2 Bass kernel for nn_Decoder_22196390985918 (SPADE-style decoder).

Sharding: 8 cores = (batch b in 0..3) x (H-half in 0..1). Each core computes
out[b, :, h0:h0+64, :] for h0 = 64*(core%2).

The [B, 512, H, W] "middle" tensor (masked scatter of per-region style
vectors mu[b,j,:]) is never materialized: conv(middle) collapses to a conv
over the 5 one-hot region masks sel_j with per-batch tap tables
G[j, cc, tap] = sum_k Wconv[cc, k, tap] * mu[b, j, k], i.e. one K=45 matmul
per output tile on top of the 9 K=128 SPADE tap matmuls.

All heavy matmul operands are bf16; weight transposes and the sigmoid
blending factors are folded on the host; every large DMA is a plain 2D
transfer with per-partition-contiguous spans. The PE stream is ordered to
avoid idling while weights land: the SPADE tap matmuls for chunks 0-2 run
first (their psum groups stay open), then mu (fc bias folded in as a K=1
matmul row), the G tables, a 9-matmul partition-shuffle that builds selG in
psum, and only then the selG "close" matmuls. gamma/beta leave PSUM via two
half-height ACTs (the beta ACT reads PSUM partitions 64:128 and writes
0:64). Relu/compare epilogues of the aux chunks run on DVE/Pool via
tensor_scalar (is_lt / add+max) to keep the ACT engine off the critical
path. Instance-norm stats come from a bf16 copy of the full image.
"""
import os as _os

import numpy as np
import ml_dtypes

import concourse.bacc as bacc
import concourse.bass as bass
import concourse.mybir as mybir
import concourse.tile as tile
from concourse.bass_utils import run_bass_kernel_spmd

dt = mybir.dt
F32 = dt.float32
BF16 = dt.bfloat16
AF = mybir.ActivationFunctionType
ALU = mybir.AluOpType
NPBF = ml_dtypes.bfloat16

B, C, H, W, F, L, NH = 4, 64, 128, 128, 5, 512, 128
GW = 130                    # padded grid width  (image col = grid col - 1)
SR = 66                     # seg/sel/actv grid rows (image row = h0 - 1 + r)
MR = 68                     # mask grid rows (image row = h0 - 2 + r)
SEG_N = SR * GW             # 8580
MASK_N = MR * GW            # 8840
ROWS = 64                   # output rows per core
NCH = 16                    # main conv chunks (4 rows x 128 cols, N=512)
ACH = 22                    # shared conv chunks (3 rows x 128 cols, N=384)
NCORES = 8
CNTC = 17                   # cnt/sel chunks of 512 cols over SEG_N
PKF = 25                    # f32 const pack cols
# bf16 pack: u5(45) | sswT(128) | codesT(20) | zeros(132)
PKB = 325
# small pack on 8 partitions: fcb rows(20*128) | Et(9*45) | one
PKE_ET = 20 * 128
PKE = PKE_ET + 9 * 45 + 1


def _build_nc():
    lvl = int(_os.environ.get("KSEC", "9"))
    nc = bacc.Bacc()

    # ---- per-core DRAM inputs -------------------------------------------
    xq_d = nc.dram_tensor("xq", [128, ROWS * W], BF16, kind="ExternalInput")
    segg = nc.dram_tensor("segg", [F, SEG_N + 264], BF16, kind="ExternalInput")
    maskg = nc.dram_tensor("maskg", [3, MASK_N + 264], BF16,
                           kind="ExternalInput")
    fcwT_d = nc.dram_tensor("fcwT", [128, F * 4 * L], BF16,
                            kind="ExternalInput")
    wct_d = nc.dram_tensor("wct", [L, 9 * 128], BF16, kind="ExternalInput")
    spT_d = nc.dram_tensor("spT", [NH, 9 * 128], BF16, kind="ExternalInput")
    pkf_d = nc.dram_tensor("pkf", [128, PKF], F32, kind="ExternalInput")
    pkb_d = nc.dram_tensor("pkb", [128, PKB], BF16, kind="ExternalInput")
    pke_d = nc.dram_tensor("pke", [8, PKE], BF16, kind="ExternalInput")
    out_d = nc.dram_tensor("out", [C, NCH, 512], F32, kind="ExternalOutput")

    segp = segg[:].ap[0][0]     # dram row stride (elements)
    maskp = maskg[:].ap[0][0]

    with tile.TileContext(nc) as tc:
        with (
            tc.tile_pool(name="const", bufs=1) as cst,
            tc.tile_pool(name="gg", bufs=3) as ggp,
            tc.tile_pool(name="bb", bufs=3) as bbp,
            tc.tile_pool(name="ot", bufs=3) as otp,
            tc.tile_pool(name="pmain", bufs=4, space="PSUM") as pmain,
            tc.tile_pool(name="paux", bufs=2, space="PSUM") as paux,
            tc.tile_pool(name="gpsp", bufs=2, space="PSUM") as gpsp,
        ):
            # ---- const packs (first on sync queue) ----------------------
            pkf = cst.tile([128, PKF], F32)
            nc.sync.dma_start(out=pkf[:], in_=pkf_d[:])
            pkb = cst.tile([128, PKB], BF16)
            nc.sync.dma_start(out=pkb[:], in_=pkb_d[:])
            pke = cst.tile([8, PKE], BF16)
            nc.sync.dma_start(out=pke[:], in_=pke_d[:])
            bias1g = pkf[0:64, 20:21]
            bias1b = pkf[0:64, 21:22]
            ssb_t = pkf[:, 22:23]
            ones45 = pkf[0:45, 19:20]
            hal_t = pkf[:, 23:25]
            u5r = pkb[0:45, 0:45]
            zsb = pkb[:, 193:325]
            eps_t = pkf[0:64, 18:19]
            ones_bf = pke[0:1, PKE - 1:PKE]
            sswT = pkb[0:27, 45:173]
            codesT = pkb[:, 173:193].rearrange("p (l j) -> p l j", l=4)

            # ---- fcwT: plain 2D loads, j0/j2/j4 scalar, j1/j3 sync ------
            ftall = cst.tile([128, F, 4, L], BF16)
            def load_ft(j):
                eng = nc.scalar if j % 2 == 0 else nc.sync
                eng.dma_start(
                    out=ftall[:, j, :, :].rearrange("p l k -> p (l k)"),
                    in_=fcwT_d[:, j * 4 * L:(j + 1) * 4 * L])



            spT_f = cst.tile([128, 9 * 128], BF16)
            nc.sync.dma_start(out=spT_f[:], in_=spT_d[:])
            spT = spT_f[:].rearrange("p (t c) -> p t c", t=9)
            for j in range(F):
                load_ft(j)
            sel45 = cst.tile([45, SEG_N], BF16)
            for ty in range(3):
                src = bass.AP(tensor=segg[:].tensor, offset=ty * GW,
                              ap=[[1, 3], [segp, F], [1, SEG_N]])
                nc.gpsimd.dma_start(out=sel45[15 * ty:15 * ty + 15, :],
                                    in_=src)
            mask27 = cst.tile([27, MASK_N], BF16)
            for ty in range(3):
                src = bass.AP(tensor=maskg[:].tensor, offset=ty * GW,
                              ap=[[1, 3], [maskp, 3], [1, MASK_N]])
                nc.gpsimd.dma_start(out=mask27[9 * ty:9 * ty + 9, :], in_=src)
            wct_sb = cst.tile([128, 4, 9 * 128], BF16)
            nc.gpsimd.dma_start(
                out=wct_sb[:].rearrange("p a b -> p (a b)"),
                in_=bass.AP(tensor=wct_d[:].tensor, offset=0,
                            ap=[[1152, 128], [128 * 1152, 4], [1, 1152]]))
            wcts = [wct_sb[:, kb, :].rearrange("p (t c) -> p t c", t=9)
                    for kb in range(4)]
            # x in SBUF once, bf16 [128, 8192]: own half on partitions 0:64
            # (stats + epilogue), other half on 64:128 (stats only)
            xq = cst.tile([128, ROWS * W], BF16)
            for h in range(2):
                nc.scalar.dma_start(out=xq[:, h * 4096:(h + 1) * 4096],
                                    in_=xq_d[:, h * 4096:(h + 1) * 4096])

            # ---- aux tiles + emitters -----------------------------------
            t_sb = cst.tile([45, SEG_N], BF16)
            actv = cst.tile([NH, SR, GW], BF16)
            # zero border cols 0 and 129 of actv (vector, only needs zsb)
            bord = actv[:, :, 0:1]
            nc.vector.tensor_copy(
                bass.AP(tensor=bord.tensor, offset=bord.offset,
                        ap=[bord.ap[0], [GW, SR], [GW - 1, 2]]),
                zsb.rearrange("p (a b) -> p a b", a=SR))
            m3 = mask27[:].rearrange("p (r c) -> p r c", c=GW)
            s3 = sel45[:].rearrange("p (r c) -> p r c", c=GW)

            segchunks = []
            off = 0
            while off < SEG_N:
                n = min(512, SEG_N - off)
                segchunks.append((off, n))
                off += n

            def cnt_chunk(c):
                off, n = segchunks[c]
                pc = paux.tile([45, 512], F32, tag="aux", name=f"cnt{c}")
                nc.tensor.matmul(pc[:, 0:n], u5r, sel45[:, off:off + n],
                                 start=True, stop=True)
                # t = relu(1 - cnt); sel *= t  (exact in bf16)
                nc.scalar.activation(t_sb[:, off:off + n], pc[:, 0:n],
                                     AF.Relu, bias=ones45, scale=-1.0)
                nc.vector.tensor_mul(sel45[:, off:off + n],
                                     sel45[:, off:off + n],
                                     t_sb[:, off:off + n])

            def shared_chunk(a):
                r = 3 * a
                psh = paux.tile([NH, 3, 128], F32, tag="aux", name=f"sh{a}")
                nc.tensor.matmul(psh[:], sswT, m3[:, r:r + 3, 0:128],
                                 start=True, stop=True)
                nc.scalar.activation(actv[:, r:r + 3, 1:129], psh[:],
                                     AF.Relu, bias=ssb_t, scale=1.0)
                if a == 0:
                    nc.vector.tensor_scalar_mul(actv[:, 0, :], actv[:, 0, :],
                                                hal_t[:, 0:1])
                elif a == ACH - 1:
                    nc.vector.tensor_scalar_mul(actv[:, SR - 1, :],
                                                actv[:, SR - 1, :],
                                                hal_t[:, 1:2])

            cnt_done = [0]
            sh_done = [0]

            def aux_for(i):
                need_cnt = min(CNTC, (520 * i + 518) // 512 + 1)
                need_sh = min(ACH, (4 * i + 6) // 3 + 1)
                while cnt_done[0] < need_cnt:
                    cnt_chunk(cnt_done[0])
                    cnt_done[0] += 1
                while sh_done[0] < need_sh:
                    shared_chunk(sh_done[0])
                    sh_done[0] += 1

            # ---- main conv pieces ---------------------------------------
            pms = {}

            def taps_chunk(i):
                pm = pmain.tile([128, 4, 128], F32, tag="pm", name=f"pm{i}")
                pms[i] = pm
                for t in range(9):
                    ty, tx = divmod(t, 3)
                    nc.tensor.matmul(
                        pm[:], spT[:, t, :],
                        actv[:, 4 * i + ty:4 * i + ty + 4, tx:tx + 128],
                        start=(t == 0), stop=False)

            def close_chunk(i, selG):
                nc.tensor.matmul(pms[i][:], selG,
                                 s3[:, 4 * i:4 * i + 4, 0:128],
                                 start=False, stop=True)

            def epi_chunk(i, rstd, nbias):
                pm = pms.pop(i)
                pmf = pm[:].rearrange("p t c -> p (t c)")
                gg = ggp.tile([C, 512], F32, tag="gg", name=f"gg{i}")
                nc.scalar.activation(gg[:], pmf[0:64, :], AF.Identity,
                                     bias=bias1g, scale=1.0)
                bb = bbp.tile([C, 512], F32, tag="bb", name=f"bb{i}")
                nc.scalar.activation(bb[:], pmf[64:128, :], AF.Identity,
                                     bias=bias1b, scale=1.0)
                xnt = otp.tile([C, 512], F32, tag="ot", name=f"xnt{i}")
                nc.gpsimd.tensor_scalar(xnt[:],
                                        xq[0:64, i * 512:(i + 1) * 512],
                                        rstd, nbias,
                                        op0=ALU.mult, op1=ALU.add)
                nc.gpsimd.tensor_mul(xnt[:], xnt[:], gg[:])
                nc.vector.tensor_add(xnt[:], xnt[:], bb[:])
                nc.sync.dma_start(out=out_d[:, i, :], in_=xnt[:])

            # ---- PE stream ----------------------------------------------
            # cnt chunks first (need only sel45, the first grid to land),
            # then shared (mask27), taps 0-2, then the cnt tail
            for c in range(9):
                cnt_chunk(c)
            for a in range(5):
                shared_chunk(a)
            sh_done[0] = 5
            for i in range(3):
                taps_chunk(i)
            for c in range(9, CNTC):
                cnt_chunk(c)
            cnt_done[0] = CNTC


            # mu: 100 small matmuls (fc bias folded in as K=1 row)
            pz = gpsp.tile([128, 4, F], F32, tag="gps", name="pz")
            for j in range(F):
                for kb in range(4):
                    for lb in range(4):
                        nc.tensor.matmul(
                            pz[:, kb, j:j + 1],
                            ftall[:, j, lb, kb * 128:(kb + 1) * 128],
                            codesT[:, lb, j:j + 1],
                            start=(lb == 0), stop=False)
                    fcbcol = (j * 4 + kb) * 128
                    nc.tensor.matmul(pz[:, kb, j:j + 1],
                                     pke[0:1, fcbcol:fcbcol + 128],
                                     ones_bf,
                                     start=False, stop=True)
            muT = cst.tile([128, 4, F], BF16)
            nc.scalar.activation(muT[:], pz[:], AF.Relu)

            # G matmuls (group-sequential so gpsp needs only 2 banks)
            gstage = cst.tile([F, 9, 128], BF16)
            for g in range(3):
                gp = gpsp.tile([F, 3, 128], F32, tag="gps", name=f"gps{g}")
                for kb in range(4):
                    nc.tensor.matmul(gp[:], muT[:, kb, :],
                                     wcts[kb][:, 3 * g:3 * g + 3, :],
                                     start=(kb == 0), stop=(kb == 3))
                nc.scalar.activation(gstage[:, 3 * g:3 * g + 3, :],
                                     gp[:], AF.Copy)
            # selG[5t+j, cc] = gstage[j, t, cc] via 9 accumulating
            # partition-shuffle matmuls (lhsT Et[j, 5t+j] = 1)
            selG_ps = gpsp.tile([45, 128], F32, tag="gps", name="selG_ps")
            for t in range(9):
                etcol = PKE_ET + 45 * t
                nc.tensor.matmul(selG_ps[:], pke[0:5, etcol:etcol + 45],
                                 gstage[:, t, :],
                                 start=(t == 0), stop=(t == 8))
            selG_t = cst.tile([45, 128], BF16)
            nc.scalar.activation(selG_t[:], selG_ps[:], AF.Copy)
            selG = selG_t[:]

            # instance-norm stats: both image halves at once on 128
            # partitions, then closed-form merge of the two halves
            stats_t = cst.tile([128, 16, 6], F32)
            for q in range(16):
                nc.vector.bn_stats(out=stats_t[:, q, :],
                                   in_=xq[:, q * 512:(q + 1) * 512])
            mv = cst.tile([128, 2], F32)
            nc.vector.bn_aggr(out=mv[:], in_=stats_t[:])
            mvhi = cst.tile([C, 2], F32)
            nc.sync.dma_start(out=mvhi[:], in_=mv[64:128, :])
            # mean = (m0+m1)/2 ; var = (v0+v1)/2 + ((m0-m1)/2)^2
            mean = cst.tile([C, 1], F32)
            nc.vector.tensor_add(mean[:], mv[0:64, 0:1], mvhi[:, 0:1])
            nc.vector.tensor_scalar_mul(mean[:], mean[:], 0.5)
            md = cst.tile([C, 1], F32)
            nc.vector.tensor_sub(md[:], mv[0:64, 0:1], mvhi[:, 0:1])
            nc.vector.tensor_scalar_mul(md[:], md[:], 0.5)
            nc.vector.tensor_mul(md[:], md[:], md[:])
            var = cst.tile([C, 1], F32)
            nc.vector.tensor_add(var[:], mv[0:64, 1:2], mvhi[:, 1:2])
            nc.vector.tensor_scalar(var[:], var[:], 0.5, None, op0=ALU.mult)
            nc.vector.tensor_add(var[:], var[:], md[:])
            sd = cst.tile([C, 1], F32)
            nc.scalar.activation(sd[:], var[:], AF.Sqrt,
                                 bias=eps_t, scale=1.0)
            rstd = cst.tile([C, 1], F32)
            nc.vector.reciprocal(rstd[:], sd[:])
            nbias = cst.tile([C, 1], F32)
            nc.vector.tensor_mul(nbias[:], mean[:], rstd[:])
            nc.vector.tensor_scalar_mul(nbias[:], nbias[:], -1.0)

            if lvl >= 6:
                for i in range(3):
                    close_chunk(i, selG)
                epi_chunk(0, rstd[:], nbias[:])
                for i in range(3, NCH):
                    aux_for(i)
                    taps_chunk(i)
                    close_chunk(i, selG)
                    epi_chunk(i - 2, rstd[:], nbias[:])
                while cnt_done[0] < CNTC:
                    cnt_chunk(cnt_done[0])
                    cnt_done[0] += 1
                while sh_done[0] < ACH:
                    shared_chunk(sh_done[0])
                    sh_done[0] += 1
                epi_chunk(NCH - 2, rstd[:], nbias[:])
                epi_chunk(NCH - 1, rstd[:], nbias[:])

    nc.finalize()
    return nc


_NC = None


def _make_in_maps(inputs):
    x = np.asarray(inputs["x"], dtype=np.float32)
    segmap = np.asarray(inputs["segmap"], dtype=np.float32)
    codes_vector = np.asarray(inputs["codes_vector"], dtype=np.float32)
    mask = np.asarray(inputs["mask"], dtype=np.float32)
    fc_w = np.asarray(inputs["fc_w"], dtype=np.float32)
    fc_b = np.asarray(inputs["fc_b"], dtype=np.float32)
    conv_gamma_w = np.asarray(inputs["conv_gamma_w"], dtype=np.float32)
    conv_gamma_b = np.asarray(inputs["conv_gamma_b"], dtype=np.float32)
    conv_beta_w = np.asarray(inputs["conv_beta_w"], dtype=np.float32)
    conv_beta_b = np.asarray(inputs["conv_beta_b"], dtype=np.float32)
    spade_shared_w = np.asarray(inputs["spade_shared_w"], dtype=np.float32)
    spade_shared_b = np.asarray(inputs["spade_shared_b"], dtype=np.float32)
    spade_gamma_w = np.asarray(inputs["spade_gamma_w"], dtype=np.float32)
    spade_gamma_b = np.asarray(inputs["spade_gamma_b"], dtype=np.float32)
    spade_beta_w = np.asarray(inputs["spade_beta_w"], dtype=np.float32)
    spade_beta_b = np.asarray(inputs["spade_beta_b"], dtype=np.float32)
    blending_gamma = np.asarray(inputs["blending_gamma"], dtype=np.float32)
    blending_beta = np.asarray(inputs["blending_beta"], dtype=np.float32)

    ga = 1.0 / (1.0 + np.exp(-float(blending_gamma[0])))
    ba = 1.0 / (1.0 + np.exp(-float(blending_beta[0])))

    # combined conv weights, blend folded in, transposed to lhsT layouts
    wc = np.concatenate([ga * conv_gamma_w, ba * conv_beta_w], axis=0)
    wct = wc.transpose(1, 2, 3, 0).reshape(L, 9 * 128)        # [k,(t,cc)]
    sp = np.concatenate([(1.0 - ga) * spade_gamma_w,
                         (1.0 - ba) * spade_beta_w], axis=0)
    spT = sp.transpose(1, 2, 3, 0).reshape(NH, 9 * 128)       # [nh,(t,cc)]
    sswT = spade_shared_w.transpose(0, 2, 3, 1).reshape(NH, 27).T  # [27,nh]
    # fcwT host layout: [p(128), j, lb, k] so each partition's data is one
    # contiguous DRAM span (descriptor-cheap 2D DMA)
    fcwT = np.ascontiguousarray(
        fc_w.transpose(0, 2, 1).reshape(F, 4, 128, L)
        .transpose(2, 0, 1, 3).reshape(128, F * 4 * L))

    # f32 const pack: (20 unused) | bias1g | bias1b | ssb | hal(2)
    pkf = np.zeros((128, PKF), np.float32)
    pkf[:, 19] = 1.0
    pkf[0:64, 20] = ga * conv_gamma_b + (1.0 - ga) * spade_gamma_b + 1.0
    pkf[0:64, 21] = ba * conv_beta_b + (1.0 - ba) * spade_beta_b
    pkf[:, 22] = spade_shared_b
    u5 = np.kron(np.eye(9, dtype=np.float32),
                 np.tril(np.ones((F, F), np.float32), -1))
    # Et[t]: [5, 45] with Et[j, 5t+j] = 1 (partition shuffle for selG)
    et = np.zeros((9, F, 45), np.float32)
    for t in range(9):
        for j in range(F):
            et[t, j, F * t + j] = 1.0

    shared = {
        "fcwT": fcwT.astype(NPBF),
        "wct": np.ascontiguousarray(wct).astype(NPBF),
        "spT": np.ascontiguousarray(spT).astype(NPBF),
    }

    in_maps = []
    for c in range(NCORES):
        b, half = divmod(c, 2)
        h0 = half * ROWS
        segp = np.zeros((F, SEG_N + 264), NPBF)
        segp2 = np.zeros((F, SR, GW), np.float32)
        r_lo, r_hi = h0 - 1, h0 + ROWS + 1  # exclusive
        s_lo, s_hi = max(r_lo, 0), min(r_hi, H)
        segp2[:, s_lo - r_lo:s_hi - r_lo, 1:129] = segmap[b, :, s_lo:s_hi, :]
        segp[:, 0:SEG_N] = segp2.reshape(F, -1).astype(NPBF)
        maskp = np.zeros((3, MASK_N + 264), NPBF)
        maskp2 = np.zeros((3, MR, GW), np.float32)
        m_lo, m_hi = h0 - 2, h0 + ROWS + 2
        ms_lo, ms_hi = max(m_lo, 0), min(m_hi, H)
        maskp2[:, ms_lo - m_lo:ms_hi - m_lo, 1:129] = mask[b, :, ms_lo:ms_hi, :]
        maskp[:, 0:MASK_N] = maskp2.reshape(3, -1).astype(NPBF)
        pkfc = pkf.copy()
        pkfc[:, 23] = 0.0 if h0 == 0 else 1.0
        pkfc[:, 24] = 0.0 if h0 + ROWS == H else 1.0
        pkb = np.zeros((128, PKB), NPBF)
        pkb[0:45, 0:45] = u5.astype(NPBF)
        pkb[0:27, 45:173] = sswT.astype(NPBF)
        pkb[:, 173:193] = (codes_vector[b].T.reshape(4, 128, F)
                           .transpose(1, 0, 2).reshape(128, 20).astype(NPBF))
        pke = np.zeros((8, PKE), NPBF)
        pke[0, 0:PKE_ET] = fc_b.reshape(F * 4 * 128).astype(NPBF)
        pke[0:F, PKE_ET:PKE - 1] = et.transpose(1, 0, 2).reshape(F, 9 * 45)
        pke[0, PKE - 1] = 1.0
        oh0 = ROWS - h0
        xsw = np.concatenate([x[b, :, h0:h0 + ROWS, :].reshape(C, ROWS * W),
                              x[b, :, oh0:oh0 + ROWS, :].reshape(C, ROWS * W)],
                             axis=0)
        in_maps.append(dict(
            shared,
            xq=np.ascontiguousarray(xsw).astype(NPBF),
            pkf=pkfc,
            pkb=pkb,
            pke=pke,
            segg=np.ascontiguousarray(segp),
            maskg=np.ascontiguousarray(maskp),
        ))
    return in_maps


def kernel(**inputs):
    global _NC
    if _NC is None:
        _NC = _build_nc()
    in_maps = _make_in_maps(inputs)
    res = run_bass_kernel_spmd(_NC, in_maps, list(range(NCORES)))

    out = np.empty((B, C, H, W), np.float32)
    for c in range(NCORES):
        b, half = divmod(c, 2)
        h0 = half * ROWS
        out[b, :, h0:h0 + ROWS, :] = res.results[c]["out"].reshape(C, ROWS, W)
    return out


# revision 32
# speedup vs baseline: 1.2039x; 1.0642x over previous
"""Trainium2 Bass kernel for nn_Decoder_22196390985918 (SPADE-style decoder).

Sharding: 8 cores = (batch b in 0..3) x (H-half in 0..1). Each core computes
out[b, :, h0:h0+64, :] for h0 = 64*(core%2).

The [B, 512, H, W] "middle" tensor (masked scatter of per-region style
vectors mu[b,j,:]) is never materialized: conv(middle) collapses to a conv
over the 5 one-hot region masks sel_j with per-batch tap tables
G[j, cc, tap] = sum_k Wconv[cc, k, tap] * mu[b, j, k], i.e. one K=45 matmul
per output tile on top of the 9 K=128 SPADE tap matmuls.

All heavy matmul operands are bf16; weight transposes and the sigmoid
blending factors are folded on the host; every large DMA is a plain 2D
transfer with per-partition-contiguous spans. The PE stream is ordered to
avoid idling while weights land: the SPADE tap matmuls for chunks 0-2 run
first (their psum groups stay open), then mu (fc bias folded in as a K=1
matmul row), the G tables, a 9-matmul partition-shuffle that builds selG in
psum, and only then the selG "close" matmuls. gamma/beta leave PSUM via two
half-height ACTs (the beta ACT reads PSUM partitions 64:128 and writes
0:64). Relu/compare epilogues of the aux chunks run on DVE/Pool via
tensor_scalar (is_lt / add+max) to keep the ACT engine off the critical
path. Instance-norm stats come from a bf16 copy of the full image.
"""
import os as _os

import numpy as np
import ml_dtypes

import concourse.bacc as bacc
import concourse.bass as bass
import concourse.mybir as mybir
import concourse.tile as tile
from concourse.bass_utils import run_bass_kernel_spmd

dt = mybir.dt
F32 = dt.float32
BF16 = dt.bfloat16
AF = mybir.ActivationFunctionType
ALU = mybir.AluOpType
NPBF = ml_dtypes.bfloat16

B, C, H, W, F, L, NH = 4, 64, 128, 128, 5, 512, 128
GW = 130                    # padded grid width  (image col = grid col - 1)
SR = 66                     # seg/sel/actv grid rows (image row = h0 - 1 + r)
MR = 68                     # mask grid rows (image row = h0 - 2 + r)
SEG_N = SR * GW             # 8580
MASK_N = MR * GW            # 8840
ROWS = 64                   # output rows per core
NCH = 16                    # main conv chunks (4 rows x 128 cols, N=512)
ACH = 22                    # shared conv chunks (3 rows x 128 cols, N=384)
NCORES = 8
CNTC = 17                   # cnt/sel chunks of 512 cols over SEG_N
PKF = 25                    # f32 const pack cols
# bf16 pack: u5(45) | sswT(128) | codesT(20) | zeros(132)
PKB = 325
# small pack on 8 partitions: fcb rows(20*128) | Et(9*45) | one
PKE_ET = 20 * 128
PKE = PKE_ET + 9 * 45 + 1


def _build_nc():
    lvl = int(_os.environ.get("KSEC", "9"))
    nc = bacc.Bacc()

    # ---- per-core DRAM inputs -------------------------------------------
    xq_d = nc.dram_tensor("xq", [128, ROWS * W], BF16, kind="ExternalInput")
    segg = nc.dram_tensor("segg", [F, SEG_N + 264], BF16, kind="ExternalInput")
    maskg = nc.dram_tensor("maskg", [3, MASK_N + 264], BF16,
                           kind="ExternalInput")
    fcwT_d = nc.dram_tensor("fcwT", [128, F * 4 * L], BF16,
                            kind="ExternalInput")
    wct_d = nc.dram_tensor("wct", [L, 9 * 128], BF16, kind="ExternalInput")
    spT_d = nc.dram_tensor("spT", [NH, 9 * 128], BF16, kind="ExternalInput")
    pkf_d = nc.dram_tensor("pkf", [128, PKF], F32, kind="ExternalInput")
    pkb_d = nc.dram_tensor("pkb", [128, PKB], BF16, kind="ExternalInput")
    pke_d = nc.dram_tensor("pke", [8, PKE], BF16, kind="ExternalInput")
    out_d = nc.dram_tensor("out", [C, NCH, 512], F32, kind="ExternalOutput")

    segp = segg[:].ap[0][0]     # dram row stride (elements)
    maskp = maskg[:].ap[0][0]

    with tile.TileContext(nc) as tc:
        with (
            tc.tile_pool(name="const", bufs=1) as cst,
            tc.tile_pool(name="gg", bufs=3) as ggp,
            tc.tile_pool(name="bb", bufs=3) as bbp,
            tc.tile_pool(name="ot", bufs=3) as otp,
            tc.tile_pool(name="pmain", bufs=4, space="PSUM") as pmain,
            tc.tile_pool(name="paux", bufs=2, space="PSUM") as paux,
            tc.tile_pool(name="gpsp", bufs=2, space="PSUM") as gpsp,
        ):
            # ---- const packs (first on sync queue) ----------------------
            pkf = cst.tile([128, PKF], F32)
            nc.sync.dma_start(out=pkf[:], in_=pkf_d[:])
            pkb = cst.tile([128, PKB], BF16)
            nc.sync.dma_start(out=pkb[:], in_=pkb_d[:])
            pke = cst.tile([8, PKE], BF16)
            nc.sync.dma_start(out=pke[:], in_=pke_d[:])
            bias1g = pkf[0:64, 20:21]
            bias1b = pkf[0:64, 21:22]
            ssb_t = pkf[:, 22:23]
            ones45 = pkf[0:45, 19:20]
            hal_t = pkf[:, 23:25]
            u5r = pkb[0:45, 0:45]
            zsb = pkb[:, 193:325]
            eps_t = pkf[0:64, 18:19]
            ones_bf = pke[0:1, PKE - 1:PKE]
            sswT = pkb[0:27, 45:173]
            codesT = pkb[:, 173:193].rearrange("p (l j) -> p l j", l=4)

            # ---- fcwT: plain 2D loads, j0/j2/j4 scalar, j1/j3 sync ------
            ftall = cst.tile([128, F, 4, L], BF16)
            def load_ft(j):
                eng = nc.scalar if j % 2 == 0 else nc.sync
                eng.dma_start(
                    out=ftall[:, j, :, :].rearrange("p l k -> p (l k)"),
                    in_=fcwT_d[:, j * 4 * L:(j + 1) * 4 * L])



            spT_f = cst.tile([128, 9 * 128], BF16)
            nc.sync.dma_start(out=spT_f[:], in_=spT_d[:])
            spT = spT_f[:].rearrange("p (t c) -> p t c", t=9)
            for j in range(F):
                load_ft(j)
            sel45 = cst.tile([45, SEG_N], BF16)
            for ty in range(3):
                src = bass.AP(tensor=segg[:].tensor, offset=ty * GW,
                              ap=[[1, 3], [segp, F], [1, SEG_N]])
                nc.gpsimd.dma_start(out=sel45[15 * ty:15 * ty + 15, :],
                                    in_=src)
            mask27 = cst.tile([27, MASK_N], BF16)
            for ty in range(3):
                src = bass.AP(tensor=maskg[:].tensor, offset=ty * GW,
                              ap=[[1, 3], [maskp, 3], [1, MASK_N]])
                nc.gpsimd.dma_start(out=mask27[9 * ty:9 * ty + 9, :], in_=src)
            wct_sb = cst.tile([128, 4, 9 * 128], BF16)
            nc.sync.dma_start(
                out=wct_sb[:].rearrange("p a b -> p (a b)"),
                in_=bass.AP(tensor=wct_d[:].tensor, offset=0,
                            ap=[[1152, 128], [128 * 1152, 4], [1, 1152]]))
            wcts = [wct_sb[:, kb, :].rearrange("p (t c) -> p t c", t=9)
                    for kb in range(4)]
            # x in SBUF once, bf16 [128, 8192]: own half on partitions 0:64
            # (stats + epilogue), other half on 64:128 (stats only)
            xq = cst.tile([128, ROWS * W], BF16)
            for h in range(2):
                nc.scalar.dma_start(out=xq[:, h * 4096:(h + 1) * 4096],
                                    in_=xq_d[:, h * 4096:(h + 1) * 4096])

            # ---- aux tiles + emitters -----------------------------------
            t_sb = cst.tile([45, SEG_N], BF16)
            actv = cst.tile([NH, SR, GW], BF16)
            # zero border cols 0 and 129 of actv (vector, only needs zsb)
            bord = actv[:, :, 0:1]
            nc.vector.tensor_copy(
                bass.AP(tensor=bord.tensor, offset=bord.offset,
                        ap=[bord.ap[0], [GW, SR], [GW - 1, 2]]),
                zsb.rearrange("p (a b) -> p a b", a=SR))
            m3 = mask27[:].rearrange("p (r c) -> p r c", c=GW)
            s3 = sel45[:].rearrange("p (r c) -> p r c", c=GW)

            segchunks = []
            off = 0
            while off < SEG_N:
                n = min(512, SEG_N - off)
                segchunks.append((off, n))
                off += n

            def cnt_chunk(c):
                off, n = segchunks[c]
                pc = paux.tile([45, 512], F32, tag="aux", name=f"cnt{c}")
                nc.tensor.matmul(pc[:, 0:n], u5r, sel45[:, off:off + n],
                                 start=True, stop=True)
                # t = relu(1 - cnt); sel *= t  (exact in bf16)
                nc.scalar.activation(t_sb[:, off:off + n], pc[:, 0:n],
                                     AF.Relu, bias=ones45, scale=-1.0)
                nc.vector.tensor_mul(sel45[:, off:off + n],
                                     sel45[:, off:off + n],
                                     t_sb[:, off:off + n])

            def shared_chunk(a):
                r = 3 * a
                psh = paux.tile([NH, 3, 128], F32, tag="aux", name=f"sh{a}")
                nc.tensor.matmul(psh[:], sswT, m3[:, r:r + 3, 0:128],
                                 start=True, stop=True)
                nc.scalar.activation(actv[:, r:r + 3, 1:129], psh[:],
                                     AF.Relu, bias=ssb_t, scale=1.0)
                if a == 0:
                    nc.vector.tensor_scalar_mul(actv[:, 0, :], actv[:, 0, :],
                                                hal_t[:, 0:1])
                elif a == ACH - 1:
                    nc.vector.tensor_scalar_mul(actv[:, SR - 1, :],
                                                actv[:, SR - 1, :],
                                                hal_t[:, 1:2])

            cnt_done = [0]
            sh_done = [0]

            def aux_for(i):
                need_cnt = min(CNTC, (520 * i + 518) // 512 + 1)
                need_sh = min(ACH, (4 * i + 6) // 3 + 1)
                while cnt_done[0] < need_cnt:
                    cnt_chunk(cnt_done[0])
                    cnt_done[0] += 1
                while sh_done[0] < need_sh:
                    shared_chunk(sh_done[0])
                    sh_done[0] += 1

            # ---- main conv pieces ---------------------------------------
            pms = {}

            def taps_chunk(i):
                pm = pmain.tile([128, 4, 128], F32, tag="pm", name=f"pm{i}")
                pms[i] = pm
                for t in range(9):
                    ty, tx = divmod(t, 3)
                    nc.tensor.matmul(
                        pm[:], spT[:, t, :],
                        actv[:, 4 * i + ty:4 * i + ty + 4, tx:tx + 128],
                        start=(t == 0), stop=False)

            def close_chunk(i, selG):
                nc.tensor.matmul(pms[i][:], selG,
                                 s3[:, 4 * i:4 * i + 4, 0:128],
                                 start=False, stop=True)

            def epi_chunk(i, rstd, nbias):
                pm = pms.pop(i)
                pmf = pm[:].rearrange("p t c -> p (t c)")
                gg = ggp.tile([C, 512], F32, tag="gg", name=f"gg{i}")
                nc.scalar.activation(gg[:], pmf[0:64, :], AF.Identity,
                                     bias=bias1g, scale=1.0)
                bb = bbp.tile([C, 512], F32, tag="bb", name=f"bb{i}")
                nc.scalar.activation(bb[:], pmf[64:128, :], AF.Identity,
                                     bias=bias1b, scale=1.0)
                xnt = otp.tile([C, 512], F32, tag="ot", name=f"xnt{i}")
                nc.gpsimd.tensor_scalar(xnt[:],
                                        xq[0:64, i * 512:(i + 1) * 512],
                                        rstd, nbias,
                                        op0=ALU.mult, op1=ALU.add)
                nc.gpsimd.tensor_mul(xnt[:], xnt[:], gg[:])
                nc.vector.tensor_add(xnt[:], xnt[:], bb[:])
                nc.sync.dma_start(out=out_d[:, i, :], in_=xnt[:])

            # ---- PE stream ----------------------------------------------
            # cnt chunks first (need only sel45, the first grid to land),
            # then shared (mask27), taps 0-2, then the cnt tail
            for c in range(9):
                cnt_chunk(c)
            for a in range(5):
                shared_chunk(a)
            sh_done[0] = 5
            for i in range(3):
                taps_chunk(i)
            for c in range(9, CNTC):
                cnt_chunk(c)
            cnt_done[0] = CNTC


            # mu: 100 small matmuls (fc bias folded in as K=1 row)
            pzfull = gpsp.tile([128, 512], F32, tag="gps", name="pz")
            pz = pzfull[:, 0:4 * F].rearrange("p (a b) -> p a b", a=4)
            for j in range(F):
                for kb in range(4):
                    for lb in range(4):
                        nc.tensor.matmul(
                            pz[:, kb, j:j + 1],
                            ftall[:, j, lb, kb * 128:(kb + 1) * 128],
                            codesT[:, lb, j:j + 1],
                            start=(lb == 0), stop=False)
                    fcbcol = (j * 4 + kb) * 128
                    nc.tensor.matmul(pz[:, kb, j:j + 1],
                                     pke[0:1, fcbcol:fcbcol + 128],
                                     ones_bf,
                                     start=False, stop=True)
            muT = cst.tile([128, 4, F], BF16)
            nc.scalar.activation(muT[:], pz, AF.Relu)

            # G matmuls (group-sequential so gpsp needs only 2 banks)
            gstage = cst.tile([F, 9, 128], BF16)
            for g in range(3):
                gp = gpsp.tile([F, 3, 128], F32, tag="gps", name=f"gps{g}")
                for kb in range(4):
                    nc.tensor.matmul(gp[:], muT[:, kb, :],
                                     wcts[kb][:, 3 * g:3 * g + 3, :],
                                     start=(kb == 0), stop=(kb == 3))
                nc.scalar.activation(gstage[:, 3 * g:3 * g + 3, :],
                                     gp[:], AF.Copy)
            # selG[5t+j, cc] = gstage[j, t, cc] via 9 accumulating
            # partition-shuffle matmuls (lhsT Et[j, 5t+j] = 1)
            selG_ps = gpsp.tile([45, 128], F32, tag="gps", name="selG_ps")
            for t in range(9):
                etcol = PKE_ET + 45 * t
                nc.tensor.matmul(selG_ps[:], pke[0:5, etcol:etcol + 45],
                                 gstage[:, t, :],
                                 start=(t == 0), stop=(t == 8))
            selG_t = cst.tile([45, 128], BF16)
            nc.scalar.activation(selG_t[:], selG_ps[:], AF.Copy)
            selG = selG_t[:]

            # instance-norm stats: both image halves at once on 128
            # partitions, then closed-form merge of the two halves
            stats_t = cst.tile([128, 16, 6], F32)
            for q in range(16):
                nc.vector.bn_stats(out=stats_t[:, q, :],
                                   in_=xq[:, q * 512:(q + 1) * 512])
            mv = cst.tile([128, 2], F32)
            nc.vector.bn_aggr(out=mv[:], in_=stats_t[:])
            mvhi = cst.tile([C, 2], F32)
            nc.sync.dma_start(out=mvhi[:], in_=mv[64:128, :])
            # mean = (m0+m1)/2 ; var = (v0+v1)/2 + ((m0-m1)/2)^2
            mean = cst.tile([C, 1], F32)
            nc.vector.tensor_add(mean[:], mv[0:64, 0:1], mvhi[:, 0:1])
            nc.vector.tensor_scalar_mul(mean[:], mean[:], 0.5)
            md = cst.tile([C, 1], F32)
            nc.vector.tensor_sub(md[:], mv[0:64, 0:1], mvhi[:, 0:1])
            nc.vector.tensor_scalar_mul(md[:], md[:], 0.5)
            nc.vector.tensor_mul(md[:], md[:], md[:])
            var = cst.tile([C, 1], F32)
            nc.vector.tensor_add(var[:], mv[0:64, 1:2], mvhi[:, 1:2])
            nc.vector.tensor_scalar(var[:], var[:], 0.5, None, op0=ALU.mult)
            nc.vector.tensor_add(var[:], var[:], md[:])
            sd = cst.tile([C, 1], F32)
            nc.scalar.activation(sd[:], var[:], AF.Sqrt,
                                 bias=eps_t, scale=1.0)
            rstd = cst.tile([C, 1], F32)
            nc.vector.reciprocal(rstd[:], sd[:])
            nbias = cst.tile([C, 1], F32)
            nc.vector.tensor_mul(nbias[:], mean[:], rstd[:])
            nc.vector.tensor_scalar_mul(nbias[:], nbias[:], -1.0)

            if lvl >= 6:
                for i in range(3):
                    close_chunk(i, selG)
                epi_chunk(0, rstd[:], nbias[:])
                for i in range(3, NCH):
                    aux_for(i)
                    taps_chunk(i)
                    close_chunk(i, selG)
                    epi_chunk(i - 2, rstd[:], nbias[:])
                while cnt_done[0] < CNTC:
                    cnt_chunk(cnt_done[0])
                    cnt_done[0] += 1
                while sh_done[0] < ACH:
                    shared_chunk(sh_done[0])
                    sh_done[0] += 1
                epi_chunk(NCH - 2, rstd[:], nbias[:])
                epi_chunk(NCH - 1, rstd[:], nbias[:])

    nc.finalize()
    return nc


_NC = None


def _make_in_maps(inputs):
    x = np.asarray(inputs["x"], dtype=np.float32)
    segmap = np.asarray(inputs["segmap"], dtype=np.float32)
    codes_vector = np.asarray(inputs["codes_vector"], dtype=np.float32)
    mask = np.asarray(inputs["mask"], dtype=np.float32)
    fc_w = np.asarray(inputs["fc_w"], dtype=np.float32)
    fc_b = np.asarray(inputs["fc_b"], dtype=np.float32)
    conv_gamma_w = np.asarray(inputs["conv_gamma_w"], dtype=np.float32)
    conv_gamma_b = np.asarray(inputs["conv_gamma_b"], dtype=np.float32)
    conv_beta_w = np.asarray(inputs["conv_beta_w"], dtype=np.float32)
    conv_beta_b = np.asarray(inputs["conv_beta_b"], dtype=np.float32)
    spade_shared_w = np.asarray(inputs["spade_shared_w"], dtype=np.float32)
    spade_shared_b = np.asarray(inputs["spade_shared_b"], dtype=np.float32)
    spade_gamma_w = np.asarray(inputs["spade_gamma_w"], dtype=np.float32)
    spade_gamma_b = np.asarray(inputs["spade_gamma_b"], dtype=np.float32)
    spade_beta_w = np.asarray(inputs["spade_beta_w"], dtype=np.float32)
    spade_beta_b = np.asarray(inputs["spade_beta_b"], dtype=np.float32)
    blending_gamma = np.asarray(inputs["blending_gamma"], dtype=np.float32)
    blending_beta = np.asarray(inputs["blending_beta"], dtype=np.float32)

    ga = 1.0 / (1.0 + np.exp(-float(blending_gamma[0])))
    ba = 1.0 / (1.0 + np.exp(-float(blending_beta[0])))

    # combined conv weights, blend folded in, transposed to lhsT layouts
    wc = np.concatenate([ga * conv_gamma_w, ba * conv_beta_w], axis=0)
    wct = wc.transpose(1, 2, 3, 0).reshape(L, 9 * 128)        # [k,(t,cc)]
    sp = np.concatenate([(1.0 - ga) * spade_gamma_w,
                         (1.0 - ba) * spade_beta_w], axis=0)
    spT = sp.transpose(1, 2, 3, 0).reshape(NH, 9 * 128)       # [nh,(t,cc)]
    sswT = spade_shared_w.transpose(0, 2, 3, 1).reshape(NH, 27).T  # [27,nh]
    # fcwT host layout: [p(128), j, lb, k] so each partition's data is one
    # contiguous DRAM span (descriptor-cheap 2D DMA)
    fcwT = np.ascontiguousarray(
        fc_w.transpose(0, 2, 1).reshape(F, 4, 128, L)
        .transpose(2, 0, 1, 3).reshape(128, F * 4 * L))

    # f32 const pack: (20 unused) | bias1g | bias1b | ssb | hal(2)
    pkf = np.zeros((128, PKF), np.float32)
    pkf[:, 19] = 1.0
    pkf[0:64, 20] = ga * conv_gamma_b + (1.0 - ga) * spade_gamma_b + 1.0
    pkf[0:64, 21] = ba * conv_beta_b + (1.0 - ba) * spade_beta_b
    pkf[:, 22] = spade_shared_b
    u5 = np.kron(np.eye(9, dtype=np.float32),
                 np.tril(np.ones((F, F), np.float32), -1))
    # Et[t]: [5, 45] with Et[j, 5t+j] = 1 (partition shuffle for selG)
    et = np.zeros((9, F, 45), np.float32)
    for t in range(9):
        for j in range(F):
            et[t, j, F * t + j] = 1.0

    shared = {
        "fcwT": fcwT.astype(NPBF),
        "wct": np.ascontiguousarray(wct).astype(NPBF),
        "spT": np.ascontiguousarray(spT).astype(NPBF),
    }

    in_maps = []
    for c in range(NCORES):
        b, half = divmod(c, 2)
        h0 = half * ROWS
        segp = np.zeros((F, SEG_N + 264), NPBF)
        segp2 = np.zeros((F, SR, GW), np.float32)
        r_lo, r_hi = h0 - 1, h0 + ROWS + 1  # exclusive
        s_lo, s_hi = max(r_lo, 0), min(r_hi, H)
        segp2[:, s_lo - r_lo:s_hi - r_lo, 1:129] = segmap[b, :, s_lo:s_hi, :]
        segp[:, 0:SEG_N] = segp2.reshape(F, -1).astype(NPBF)
        maskp = np.zeros((3, MASK_N + 264), NPBF)
        maskp2 = np.zeros((3, MR, GW), np.float32)
        m_lo, m_hi = h0 - 2, h0 + ROWS + 2
        ms_lo, ms_hi = max(m_lo, 0), min(m_hi, H)
        maskp2[:, ms_lo - m_lo:ms_hi - m_lo, 1:129] = mask[b, :, ms_lo:ms_hi, :]
        maskp[:, 0:MASK_N] = maskp2.reshape(3, -1).astype(NPBF)
        pkfc = pkf.copy()
        pkfc[:, 23] = 0.0 if h0 == 0 else 1.0
        pkfc[:, 24] = 0.0 if h0 + ROWS == H else 1.0
        pkb = np.zeros((128, PKB), NPBF)
        pkb[0:45, 0:45] = u5.astype(NPBF)
        pkb[0:27, 45:173] = sswT.astype(NPBF)
        pkb[:, 173:193] = (codes_vector[b].T.reshape(4, 128, F)
                           .transpose(1, 0, 2).reshape(128, 20).astype(NPBF))
        pke = np.zeros((8, PKE), NPBF)
        pke[0, 0:PKE_ET] = fc_b.reshape(F * 4 * 128).astype(NPBF)
        pke[0:F, PKE_ET:PKE - 1] = et.transpose(1, 0, 2).reshape(F, 9 * 45)
        pke[0, PKE - 1] = 1.0
        oh0 = ROWS - h0
        xsw = np.concatenate([x[b, :, h0:h0 + ROWS, :].reshape(C, ROWS * W),
                              x[b, :, oh0:oh0 + ROWS, :].reshape(C, ROWS * W)],
                             axis=0)
        in_maps.append(dict(
            shared,
            xq=np.ascontiguousarray(xsw).astype(NPBF),
            pkf=pkfc,
            pkb=pkb,
            pke=pke,
            segg=np.ascontiguousarray(segp),
            maskg=np.ascontiguousarray(maskp),
        ))
    return in_maps


def kernel(**inputs):
    global _NC
    if _NC is None:
        _NC = _build_nc()
    in_maps = _make_in_maps(inputs)
    res = run_bass_kernel_spmd(_NC, in_maps, list(range(NCORES)))

    out = np.empty((B, C, H, W), np.float32)
    for c in range(NCORES):
        b, half = divmod(c, 2)
        h0 = half * ROWS
        out[b, :, h0:h0 + ROWS, :] = res.results[c]["out"].reshape(C, ROWS, W)
    return out
